# revision 3
# baseline (speedup 1.0000x reference)
"""Self-contained Trainium2 (Bass/Tile) DeformConv2d kernel.

kernel(x, offset, weight) -> np.ndarray [B, Cout, H, W] float32.
Data-parallel over batch: one SPMD Bass program per NeuronCore (8 cores).

v2 vs v1: all prep (bilinear weights, gather indices, bf16 casts) moved to
host-side numpy; device does only gather + val-build + GEMM. PSUM->SBUF val
copies run on the Activation engine; the gathers use 2 SWDGE queues and a
4096-descriptor ring so descriptor-gen pipelines ahead of transfers; output
is stored bf16 and cast on host.
"""
import sys
import numpy as np
import ml_dtypes

for _p in ("/opt/trn_rl_repo",):
    if _p not in sys.path:
        sys.path.insert(0, _p)

import concourse.bass as bass
import concourse.mybir as mybir
import concourse.tile as tile
from concourse import bacc
from concourse.masks import make_identity
from concourse.bass_utils import run_bass_kernel_spmd

f32 = mybir.dt.float32
bf16 = mybir.dt.bfloat16
i16 = mybir.dt.int16
Alu = mybir.AluOpType
P = 128
BF = ml_dtypes.bfloat16


def build_dcn(C=256, Cout=256, H=64, W=64, KH=3, KW=3, CHUNKS=(8, 8, 8, 6, 2), GBUFS=4, DVE_LAST_COPY=True):
    HW = H * W
    S = HW // P
    SW = HW // 16
    NT = KH * KW
    CB = C // P
    MB = Cout // P
    assert sum(CHUNKS) == S

    nc = bacc.Bacc("TRN2", target_bir_lowering=False, debug=False,
                   dynamic_dma_scratch_size=65536, num_swdge_queues=2)

    xtb = nc.declare_dram_parameter("xtb", [HW + 2, C], bf16, isOutput=False)
    wbl = nc.declare_dram_parameter("wbl", [P, 4, NT, S], bf16, isOutput=False)
    idx0 = nc.declare_dram_parameter("idx0", [P, 2, SW], i16, isOutput=False)
    idxA = nc.declare_dram_parameter("idxA", [P, NT, SW], i16, isOutput=False)
    idxB = nc.declare_dram_parameter("idxB", [P, NT, SW], i16, isOutput=False)
    wt = nc.declare_dram_parameter("wt", [P, NT, CB, Cout], bf16, isOutput=False)
    out = nc.declare_dram_parameter("out", [Cout, HW], bf16, isOutput=True)

    with tile.TileContext(nc) as tc:
        with tc.tile_pool(name="persist", bufs=1) as pe_pool:
            wtb = pe_pool.tile([P, NT, CB, Cout], bf16, name="wtb")
            wblt = pe_pool.tile([P, 4, NT, S], f32, name="wblt")
            wblb = pe_pool.tile([P, 4, NT, S], bf16, name="wblb")
            # tap-0 index slices load first so the first gathers start
            # immediately; the rest follow.
            idx016 = pe_pool.tile([P, 2, SW], i16, name="idx016")
            idxA16 = pe_pool.tile([P, NT, SW], i16, name="idxA16")
            idxB16 = pe_pool.tile([P, NT, SW], i16, name="idxB16")
            ident = pe_pool.tile([P, P], bf16, name="ident")

            make_identity(nc, ident[:])
            nc.sync.dma_start(out=idx016[:], in_=idx0[:])
            nc.sync.dma_start(out=idxA16[:, 1:, :], in_=idxA[:, 1:, :])
            nc.sync.dma_start(out=idxB16[:, 1:, :], in_=idxB[:, 1:, :])
            nc.sync.dma_start(out=wblb[:], in_=wbl[:])
            nc.sync.dma_start(out=wtb[:], in_=wt[:])
            nc.vector.tensor_copy(out=wblt[:], in_=wblb[:])

            # overlapping-pair view of the bf16 table: row i covers elements
            # [C*i, C*i + 2C) — dma_gather elem_step=C, elem_size=2C.
            xtb_pairs = bass.AP(xtb[:].tensor, 0, [[C, HW], [1, 2 * C]])

            with (
                tc.tile_pool(name="gather", bufs=GBUFS) as g_pool,
                tc.tile_pool(name="prod", bufs=2) as pr_pool,
                tc.tile_pool(name="vout", bufs=2) as v_pool,
                tc.tile_pool(name="obuf", bufs=2) as o_pool,
                tc.tile_pool(name="psum_out", bufs=1, space="PSUM") as pso_pool,
                tc.tile_pool(name="prod2", bufs=2) as pr2_pool,
            ):
                def emit_chunk(ch, cjt, s0, psv_pool, last_ch):
                    JC = cjt * P
                    NNB = (JC + 511) // 512
                    HJT = min(cjt, 4)  # jt per val half-tile
                    NH = (cjt + HJT - 1) // HJT
                    out_ps = [
                        pso_pool.tile([P, JC], f32, space="PSUM", name=f"out_ps{_m}")
                        for _m in range(MB)
                    ]
                    for k in range(NT):
                        gA = g_pool.tile([P, cjt, 2 * C], bf16, name="gA")
                        gB = g_pool.tile([P, cjt, 2 * C], bf16, name="gB")
                        isl = slice(s0 * 8, (s0 + cjt) * 8)
                        iA = idx016[:, 0, isl] if k == 0 else idxA16[:, k, isl]
                        iB = idx016[:, 1, isl] if k == 0 else idxB16[:, k, isl]
                        nc.gpsimd.dma_gather(
                            gA[:], xtb_pairs, iA, JC, JC, 2 * C,
                            elem_step=C, queue_num=0,
                        )
                        nc.gpsimd.dma_gather(
                            gB[:], xtb_pairs, iB, JC, JC, 2 * C,
                            elem_step=C, queue_num=1,
                        )
                        # corner multiplies (DVE): pr[p, jt, n, c]
                        pieces = [
                            (gA, slice(0, C)),
                            (gA, slice(C, 2 * C)),
                            (gB, slice(0, C)),
                            (gB, slice(C, 2 * C)),
                        ]
                        if last_ch:
                            prs = [pr2_pool.tile([P, 4, C], bf16, name=f"pr2_{jt}")
                                   for jt in range(cjt)]
                        else:
                            pr = pr_pool.tile([P, cjt, 4, C], bf16, name="pr")
                            prs = [pr[:, jt] for jt in range(cjt)]
                        for jt in range(cjt):
                            s_idx = s0 + jt
                            for n, (g, csl) in enumerate(pieces):
                                nc.vector.tensor_scalar(
                                    out=prs[jt][:, n, :], in0=g[:, jt, csl],
                                    scalar1=wblt[:, n, k, s_idx:s_idx + 1],
                                    scalar2=None, op0=Alu.mult,
                                )
                        # transpose-accumulate (PE) into half-sized PSUM val
                        # tiles (bufs=2 -> consecutive taps double-buffer)
                        vsb = v_pool.tile([P, CB, JC], bf16, name="vsb")
                        for cb in range(CB):
                            for h in range(NH):
                                jts = range(h * HJT, min((h + 1) * HJT, cjt))
                                vh = psv_pool.tile([P, HJT * P], f32,
                                                   space="PSUM",
                                                   name=f"val_h{cb}")
                                for jt in jts:
                                    jo = jt - h * HJT
                                    for n in range(4):
                                        nc.tensor.matmul(
                                            out=vh[:, jo * P:(jo + 1) * P],
                                            lhsT=prs[jt][:, n,
                                                         cb * P:(cb + 1) * P],
                                            rhs=ident[:],
                                            start=(n == 0),
                                            stop=(n == 3),
                                        )
                                csl2 = slice(h * HJT * P,
                                             min((h + 1) * HJT, cjt) * P)
                                ncols = csl2.stop - csl2.start
                                if last_ch and cb == 1 and DVE_LAST_COPY:
                                    nc.vector.tensor_copy(
                                        out=vsb[:, cb, csl2],
                                        in_=vh[:, :ncols])
                                else:
                                    nc.scalar.copy(out=vsb[:, cb, csl2],
                                                   in_=vh[:, :ncols])
                        # main GEMM (PE), accumulating over taps and cb
                        for cb in range(CB):
                            for mb in range(MB):
                                for nb in range(NNB):
                                    nsl = slice(nb * 512, min((nb + 1) * 512, JC))
                                    nc.tensor.matmul(
                                        out=out_ps[mb][:, nsl],
                                        lhsT=wtb[:, k, cb, mb * P:(mb + 1) * P],
                                        rhs=vsb[:, cb, nsl],
                                        start=(k == 0 and cb == 0),
                                        stop=(k == NT - 1 and cb == CB - 1),
                                    )
                    ob = o_pool.tile([P, MB, JC], bf16, name="ob")
                    nc.vector.tensor_copy(out=ob[:, 0, :], in_=out_ps[0][:])
                    nc.scalar.copy(out=ob[:, 1, :], in_=out_ps[1][:])
                    if last_ch:
                        for mb in range(MB):
                            dst = bass.AP(out[:].tensor, mb * P * HW + s0 * P,
                                          [[HW, P], [1, JC]])
                            nc.sync.dma_start(out=dst, in_=ob[:, mb, :])
                    else:
                        # dst rows (p + 128*mb), cols [s0*P, s0*P + JC)
                        dst = bass.AP(out[:].tensor, s0 * P,
                                      [[HW, P], [P * HW, MB], [1, JC]])
                        nc.sync.dma_start(out=dst, in_=ob[:])

                s0 = 0
                with tc.tile_pool(name="psum_val", bufs=2,
                                  space="PSUM") as psv_pool:
                    for ch, cjt in enumerate(CHUNKS):
                        emit_chunk(ch, cjt, s0, psv_pool,
                                   ch == len(CHUNKS) - 1)
                        s0 += cjt

    nc.compile()
    return nc


def host_prep(x_b, offset_b, weight, H, W, KH, KW, PAD):
    """Per-core input map from one batch slice (numpy, f32)."""
    C = x_b.shape[0]
    Cout = weight.shape[0]
    HW = H * W
    S = HW // P
    SW = HW // 16
    NT = KH * KW
    CB = C // P

    xtb = np.zeros((HW + 2, C), dtype=BF)
    xtb[:HW] = x_b.reshape(C, HW).T.astype(BF)

    off = offset_b.reshape(NT, 2, HW).astype(np.float64)
    j = np.arange(HW)
    ks = np.arange(NT)
    by = j[None, :] // W - PAD + (ks // KW)[:, None]  # [k, j]
    bx = j[None, :] % W - PAD + (ks % KW)[:, None]
    py = by + off[:, 0]
    px = bx + off[:, 1]
    y0 = np.floor(py)
    x0 = np.floor(px)
    ly = (py - y0).astype(np.float32)
    lx = (px - x0).astype(np.float32)
    my0 = (y0 >= 0) & (y0 <= H - 1)
    my1 = (y0 >= -1) & (y0 <= H - 2)
    mx0 = (x0 >= 0) & (x0 <= W - 1)
    mx1 = (x0 >= -1) & (x0 <= W - 2)
    vy0 = (1.0 - ly) * my0
    vy1 = ly * my1
    ux0 = (1.0 - lx) * mx0
    ux1 = lx * mx1
    sx = np.clip(x0, 0, W - 2)
    tsh = x0 - sx
    u0 = ux0 * (tsh == 0) + ux1 * (tsh == -1)
    u1 = ux0 * (tsh == 1) + ux1 * (tsh == 0)
    wbl = np.stack([vy0 * u0, vy0 * u1, vy1 * u0, vy1 * u1])  # [4, k, j]
    yc0 = np.clip(y0, 0, H - 1)
    yc1 = np.clip(y0 + 1, 0, H - 1)
    iA = (yc0 * W + sx).astype(np.int64)  # [k, j]
    iB = (yc1 * W + sx).astype(np.int64)

    # L128 layout: j = 128*s + p -> [p, 4, k, s]
    wbl_l = np.ascontiguousarray(
        wbl.reshape(4, NT, S, P).transpose(3, 0, 1, 2)).astype(BF)

    def w16_i16(a):  # [k, j] -> [q + 16g, k, s], j = 16*s + q, replicated g
        b = a.reshape(NT, SW, 16).transpose(2, 0, 1)  # [q, k, s]
        return np.ascontiguousarray(np.tile(b, (8, 1, 1))).astype(np.int16)

    iA16, iB16 = w16_i16(iA), w16_i16(iB)
    idx0 = np.ascontiguousarray(np.stack([iA16[:, 0], iB16[:, 0]], axis=1))
    wtv = weight.reshape(Cout, CB, P, NT).transpose(2, 3, 1, 0)
    return {
        "xtb": xtb,
        "wbl": wbl_l,
        "idx0": idx0, "idxA": iA16, "idxB": iB16,
        "wt": np.ascontiguousarray(wtv).astype(BF),
    }


_NC_CACHE = {}


def _get_nc(key, **kw):
    if key not in _NC_CACHE:
        _NC_CACHE[key] = build_dcn(**kw)
    return _NC_CACHE[key]


def kernel(x, offset, weight):
    x = np.asarray(x, dtype=np.float32)
    offset = np.asarray(offset, dtype=np.float32)
    weight = np.asarray(weight, dtype=np.float32)
    B, C, H, W = x.shape
    Cout = weight.shape[0]
    KH, KW = weight.shape[2], weight.shape[3]
    PAD = 1
    assert B == 8 and C % 128 == 0 and Cout % 128 == 0
    nc = _get_nc((C, Cout, H, W, KH, KW), C=C, Cout=Cout, H=H, W=W,
                 KH=KH, KW=KW)
    in_maps = [host_prep(x[b], offset[b], weight, H, W, KH, KW, PAD)
               for b in range(B)]
    res = run_bass_kernel_spmd(nc, in_maps, list(range(B)))
    out = np.stack([
        np.asarray(res.results[b]["out"], dtype=np.float32).reshape(Cout, H, W)
        for b in range(B)
    ])
    return out
